# revision 1
# baseline (speedup 1.0000x reference)
"""Trainium2 Bass kernel for nn_Attention (B=4, N=2048, C=768, H=12, D=64).

Sharding: 8 cores = 4 batches x 2 head-groups (6 heads each).
Per core (all on-chip, bf16 matmuls, fp32 accumulation):
  qkT = (w_qk @ x^T)           [768, 2048]  (q rows pre-scaled by D^-0.5)
  v   = x @ w_v^T              [2048, 384]  (+ ones column per head)
  per head h, q-strip s(512):
    sT[kv,q] = kT_h^T-contract-d (K=64 matmuls, row-tiled head pairs)
    expS = exp(sT)  (no max subtraction: scores are O(1) by construction)
    av[q,65] = sum_kv expS^T-contract  (col 64 = softmax denominator)
    attn = av[:, :64] / av[:, 64]
    attnT via PE transpose
  y_part = attnT^T-contract @ w_p  [2048, 768]
Host: y[b] = y_part(group0) + y_part(group1) + b_proj.
"""

import sys

if "/opt/trn_rl_repo" not in sys.path:
    sys.path.insert(0, "/opt/trn_rl_repo")

import numpy as np
import ml_dtypes

import concourse.bacc as bacc
import concourse.mybir as mybir
import concourse.tile as tile
from concourse.masks import make_identity

FP32 = mybir.dt.float32
BF16 = mybir.dt.bfloat16
AF = mybir.ActivationFunctionType

DIM = 768
NUM_HEADS = 12
HEAD_DIM = 64
SCALE = HEAD_DIM ** -0.5
B, N = 4, 2048
HG = 6               # heads per core (head group)
CC = DIM // 128      # contraction chunks for qkv (6)
PAIRS = HG // 2      # head pairs per core (3)
S = N // 512         # q strips (4)
J = N // 128         # kv blocks (16)
CH = 3               # kv blocks per score psum chunk (3 banks)

_CACHED = {}


def build_core_program():
    """One NeuronCore's program (SPMD: same program on all 8 cores)."""
    nc = bacc.Bacc("TRN2", debug=False, target_bir_lowering=False, num_devices=1)

    xt_d = nc.dram_tensor("xt", [DIM, N], BF16, kind="ExternalInput")
    wqk_d = nc.dram_tensor("wqk", [DIM, DIM], BF16, kind="ExternalInput")
    wv_d = nc.dram_tensor("wv", [DIM, HG * 64], BF16, kind="ExternalInput")
    wp_d = nc.dram_tensor("wp", [HG * 64, DIM], BF16, kind="ExternalInput")
    y_d = nc.dram_tensor("y", [N, DIM], FP32, kind="ExternalOutput")

    with tile.TileContext(nc) as tc:
        with (
            tc.tile_pool(name="persist", bufs=1) as persist,
            tc.tile_pool(name="exps", bufs=18) as exps_pool,
            tc.tile_pool(name="attnt", bufs=2) as attnt_pool,
            tc.tile_pool(name="small", bufs=4) as small_pool,
            tc.tile_pool(name="ysb", bufs=3) as y_pool,
            tc.tile_pool(name="ps_score", bufs=2, space="PSUM") as ps_score,
            tc.tile_pool(name="ps_small", bufs=2, space="PSUM") as ps_small,
        ):
            # ---- persistent SBUF ----
            xT = persist.tile([128, CC, N], BF16)          # [c, n] chunked
            wqk = persist.tile([128, CC, DIM], BF16)       # [c, o] (o: 384q+384k)
            wv = persist.tile([128, CC, HG * 64], BF16)    # [c, ov]
            wp = persist.tile([128, PAIRS, DIM], BF16)     # [c', o]
            qkT = persist.tile([128, CC, N], BF16)         # [o, n]
            v = persist.tile([128, J, HG, 65], BF16)       # [kv, j, h, d+1]
            ident = persist.tile([128, 128], BF16)

            nc.sync.dma_start(out=xT, in_=xt_d.ap().rearrange("(o p) n -> p o n", p=128))
            nc.sync.dma_start(out=wqk, in_=wqk_d.ap().rearrange("(o p) n -> p o n", p=128))
            nc.sync.dma_start(out=wv, in_=wv_d.ap().rearrange("(o p) n -> p o n", p=128))
            nc.sync.dma_start(out=wp, in_=wp_d.ap().rearrange("(o p) n -> p o n", p=128))
            make_identity(nc, ident)
            nc.vector.memset(v, 1.0)  # ones in col 64 of every head survive

            def qkv_pair(p):
                """Project qT,kT (o-tiles p and 3+p) and v for head pair p."""
                for ot in (p, PAIRS + p):
                    for s in range(S):
                        ps = ps_small.tile([128, 512], FP32, tag="sm")
                        for cc in range(CC):
                            nc.tensor.matmul(
                                ps,
                                wqk[:, cc, ot * 128 : ot * 128 + 128],
                                xT[:, cc, s * 512 : s * 512 + 512],
                                start=(cc == 0), stop=(cc == CC - 1),
                            )
                        nc.vector.tensor_copy(
                            out=qkT[:, ot, s * 512 : s * 512 + 512], in_=ps
                        )
                for nt in range(J):
                    psv = ps_small.tile([128, 512], FP32, tag="sm")
                    for cc in range(CC):
                        nc.tensor.matmul(
                            psv[:, 0:128],
                            xT[:, cc, nt * 128 : nt * 128 + 128],
                            wv[:, cc, p * 128 : p * 128 + 128],
                            start=(cc == 0), stop=(cc == CC - 1),
                        )
                    for h2 in range(2):
                        nc.vector.tensor_copy(
                            out=v[:, nt, 2 * p + h2, 0:64],
                            in_=psv[:, h2 * 64 : h2 * 64 + 64],
                        )

            def proj(s, attnT):
                """y[s*512 : (s+1)*512, :] = attnT^T @ wp."""
                for nt in range(4):
                    ysb = y_pool.tile([128, DIM], FP32, tag="y")
                    for og, ow in ((0, 512), (512, 256)):
                        psy = ps_small.tile([128, 512], FP32, tag="sm")
                        for cc in range(PAIRS):
                            nc.tensor.matmul(
                                psy[:, 0:ow],
                                attnT[:, cc, nt * 128 : nt * 128 + 128],
                                wp[:, cc, og : og + ow],
                                start=(cc == 0), stop=(cc == PAIRS - 1),
                            )
                        nc.vector.tensor_copy(out=ysb[:, og : og + ow], in_=psy[:, 0:ow])
                    row = s * 512 + nt * 128
                    nc.sync.dma_start(out=y_d.ap()[row : row + 128, :], in_=ysb)

            # chunking of the 16 kv blocks into score-psum chunks
            chunks = []
            j0 = 0
            while j0 < J:
                ln = min(CH, J - j0)
                chunks.append((j0, ln))
                j0 += ln

            qkv_pair(0)
            qkv_pair(1)

            attnT_tiles = {}
            for s in range(S):
                for hp in range(PAIRS):
                    if hp == 0:
                        attnT = attnt_pool.tile([128, PAIRS, 512], BF16, tag="attnT")
                        attnT_tiles[s] = attnT
                    attnT = attnT_tiles[s]

                    hA, hB = 2 * hp, 2 * hp + 1
                    # --- QK^T + exp, chunked over kv ---
                    expS = {0: [], 1: []}
                    for (j0, ln) in chunks:
                        for h2, base in ((0, 0), (1, 64)):
                            pss = ps_score.tile([128, 512 * CH], FP32, tag="sc")
                            h = 2 * hp + h2
                            for jj in range(ln):
                                j = j0 + jj
                                nc.tensor.matmul(
                                    pss[:, jj * 512 : jj * 512 + 512],
                                    qkT[base : base + 64, PAIRS + hp,
                                        j * 128 : j * 128 + 128],
                                    qkT[base : base + 64, hp,
                                        s * 512 : s * 512 + 512],
                                    start=True, stop=True,
                                    tile_position=(base, 0),
                                )
                            et = exps_pool.tile([128, 512 * CH], BF16, tag="e")
                            nc.scalar.activation(
                                out=et[:, : 512 * ln],
                                in_=pss[:, : 512 * ln],
                                func=AF.Exp,
                            )
                            expS[h2].append((j0, ln, et))

                    # pipelined heavy PE work while ACT runs exp:
                    if s == 0 and hp < PAIRS - 1:
                        qkv_pair(hp + 1)
                    if hp == 0 and s >= 1:
                        proj(s - 1, attnT_tiles.pop(s - 1))

                    # --- AV + divide ---
                    attn_pair = small_pool.tile([128, 4, 128], BF16, tag="ap")
                    for h2 in range(2):
                        h = 2 * hp + h2
                        pav = ps_small.tile([128, 512], FP32, tag="sm")
                        for i in range(4):
                            for (j0, ln, et) in expS[h2]:
                                for jj in range(ln):
                                    j = j0 + jj
                                    nc.tensor.matmul(
                                        pav[:, i * 128 : i * 128 + 65],
                                        et[:, jj * 512 + i * 128 : jj * 512 + i * 128 + 128],
                                        v[:, j, h, :],
                                        start=(j == 0), stop=(j == J - 1),
                                    )
                        pav4 = pav.rearrange("p (r c) -> p r c", r=4)
                        rsb = small_pool.tile([128, 4], FP32, tag="r")
                        nc.vector.reciprocal(out=rsb, in_=pav4[:, :, 64])
                        nc.vector.tensor_tensor(
                            attn_pair[:, :, h2 * 64 : h2 * 64 + 64],
                            pav4[:, :, 0:64],
                            rsb[:, :, None].to_broadcast((128, 4, 64)),
                            mybir.AluOpType.mult,
                        )

                    # --- transpose pair block into attnT ---
                    for i in range(4):
                        pst = ps_small.tile([128, 512], BF16, tag="sm")
                        nc.tensor.transpose(
                            pst[:, 0:128], attn_pair[:, i, :], ident
                        )
                        nc.vector.tensor_copy(
                            out=attnT[:, hp, i * 128 : i * 128 + 128],
                            in_=pst[:, 0:128],
                        )

            proj(S - 1, attnT_tiles.pop(S - 1))

    nc.compile()
    return nc


def _host_prep(x, w_qkv, w_proj):
    """Slice/transpose/cast inputs per core. Core c = 2*b + hg."""
    bf16 = ml_dtypes.bfloat16
    in_maps = []
    for c in range(8):
        b, hg = c // 2, c % 2
        r0 = 384 * hg
        wq = w_qkv[r0 : r0 + 384] * SCALE          # [384, 768] scaled q rows
        wk = w_qkv[768 + r0 : 768 + r0 + 384]
        wv = w_qkv[1536 + r0 : 1536 + r0 + 384]
        wqk = np.concatenate([wq, wk], axis=0)     # [768, 768]
        in_maps.append({
            "xt": np.ascontiguousarray(x[b].T).astype(bf16),
            "wqk": np.ascontiguousarray(wqk.T).astype(bf16),
            "wv": np.ascontiguousarray(wv.T).astype(bf16),
            "wp": np.ascontiguousarray(w_proj[:, r0 : r0 + 384].T).astype(bf16),
        })
    return in_maps


def kernel(x, w_qkv, w_proj, b_proj):
    x = np.asarray(x, dtype=np.float32)
    w_qkv = np.asarray(w_qkv, dtype=np.float32)
    w_proj = np.asarray(w_proj, dtype=np.float32)
    b_proj = np.asarray(b_proj, dtype=np.float32)

    if "nc" not in _CACHED:
        _CACHED["nc"] = build_core_program()
    nc = _CACHED["nc"]

    in_maps = _host_prep(x, w_qkv, w_proj)

    from concourse import bass2jax

    results = bass2jax.run_bass_via_pjrt(nc, in_maps, n_cores=8)

    y = np.empty((B, N, DIM), dtype=np.float32)
    for b in range(B):
        y[b] = results[2 * b]["y"] + results[2 * b + 1]["y"] + b_proj
    return y


# revision 2
# speedup vs baseline: 67209.9702x; 67209.9702x over previous
"""Trainium2 Bass kernel for nn_Attention (B=4, N=2048, C=768, H=12, D=64).

Sharding: 8 cores = 4 batches x 2 head-groups (6 heads each).
Per core (all on-chip, bf16 matmuls, fp32 accumulation):
  qkT = (w_qk @ x^T)           [768, 2048]  (q rows pre-scaled by D^-0.5)
  v   = x @ w_v^T              [2048, 384]  (+ ones column per head)
  per head h, q-strip s(512):
    sT[kv,q] = kT_h^T-contract-d (K=64 matmuls, row-tiled head pairs)
    expS = exp(sT)  (no max subtraction: scores are O(1) by construction)
    av[q,65] = sum_kv expS^T-contract  (col 64 = softmax denominator)
    attn = av[:, :64] / av[:, 64]
    attnT via PE transpose
  y_part = attnT^T-contract @ w_p  [2048, 768]
Host: y[b] = y_part(group0) + y_part(group1) + b_proj.
"""

import sys

if "/opt/trn_rl_repo" not in sys.path:
    sys.path.insert(0, "/opt/trn_rl_repo")

import numpy as np
import ml_dtypes

import concourse.bacc as bacc
import concourse.mybir as mybir
import concourse.tile as tile
from concourse.masks import make_identity

FP32 = mybir.dt.float32
BF16 = mybir.dt.bfloat16
AF = mybir.ActivationFunctionType

DIM = 768
NUM_HEADS = 12
HEAD_DIM = 64
SCALE = HEAD_DIM ** -0.5
B, N = 4, 2048
HG = 6               # heads per core (head group)
CC = DIM // 128      # contraction chunks for qkv (6)
PAIRS = HG // 2      # head pairs per core (3)
S = N // 512         # q strips (4)
J = N // 128         # kv blocks (16)
CH = 3               # kv blocks per score psum chunk (3 banks)

_CACHED = {}


def build_core_program():
    """One NeuronCore's program (SPMD: same program on all 8 cores)."""
    nc = bacc.Bacc("TRN2", debug=False, target_bir_lowering=False, num_devices=1)

    xt_d = nc.dram_tensor("xt", [DIM, N], BF16, kind="ExternalInput")
    wqk_d = nc.dram_tensor("wqk", [DIM, DIM], BF16, kind="ExternalInput")
    wv_d = nc.dram_tensor("wv", [DIM, HG * 64], BF16, kind="ExternalInput")
    wp_d = nc.dram_tensor("wp", [HG * 64, DIM], BF16, kind="ExternalInput")
    y_d = nc.dram_tensor("y", [N, DIM], FP32, kind="ExternalOutput")

    with tile.TileContext(nc) as tc:
        with (
            tc.tile_pool(name="persist", bufs=1) as persist,
            tc.tile_pool(name="exps", bufs=18) as exps_pool,
            tc.tile_pool(name="attnt", bufs=2) as attnt_pool,
            tc.tile_pool(name="small", bufs=4) as small_pool,
            tc.tile_pool(name="ysb", bufs=3) as y_pool,
            tc.tile_pool(name="ps_score", bufs=2, space="PSUM") as ps_score,
            tc.tile_pool(name="ps_small", bufs=2, space="PSUM") as ps_small,
        ):
            # ---- persistent SBUF ----
            xT = persist.tile([128, CC, N], BF16)          # [c, n] chunked
            wqk = persist.tile([128, CC, DIM], BF16)       # [c, o] (o: 384q+384k)
            wv = persist.tile([128, CC, HG * 64], BF16)    # [c, ov]
            wp = persist.tile([128, PAIRS, DIM], BF16)     # [c', o]
            qkT = persist.tile([128, CC, N], BF16)         # [o, n]
            v = persist.tile([128, J, HG, 65], BF16)       # [kv, j, h, d+1]
            ident = persist.tile([128, 128], BF16)

            nc.sync.dma_start(out=xT, in_=xt_d.ap().rearrange("(o p) n -> p o n", p=128))
            nc.sync.dma_start(out=wqk, in_=wqk_d.ap().rearrange("(o p) n -> p o n", p=128))
            nc.sync.dma_start(out=wv, in_=wv_d.ap().rearrange("(o p) n -> p o n", p=128))
            nc.sync.dma_start(out=wp, in_=wp_d.ap().rearrange("(o p) n -> p o n", p=128))
            make_identity(nc, ident)
            nc.vector.memset(v, 1.0)  # ones in col 64 of every head survive

            def qkv_pair(p):
                """Project qT,kT (o-tiles p and 3+p) and v for head pair p."""
                for ot in (p, PAIRS + p):
                    for s in range(S):
                        ps = ps_small.tile([128, 512], FP32, tag="sm")
                        for cc in range(CC):
                            nc.tensor.matmul(
                                ps,
                                wqk[:, cc, ot * 128 : ot * 128 + 128],
                                xT[:, cc, s * 512 : s * 512 + 512],
                                start=(cc == 0), stop=(cc == CC - 1),
                            )
                        nc.vector.tensor_copy(
                            out=qkT[:, ot, s * 512 : s * 512 + 512], in_=ps
                        )
                for nt in range(J):
                    psv = ps_small.tile([128, 512], FP32, tag="sm")
                    for cc in range(CC):
                        nc.tensor.matmul(
                            psv[:, 0:128],
                            xT[:, cc, nt * 128 : nt * 128 + 128],
                            wv[:, cc, p * 128 : p * 128 + 128],
                            start=(cc == 0), stop=(cc == CC - 1),
                        )
                    for h2 in range(2):
                        nc.vector.tensor_copy(
                            out=v[:, nt, 2 * p + h2, 0:64],
                            in_=psv[:, h2 * 64 : h2 * 64 + 64],
                        )

            def proj(s, attnT):
                """y[s*512 : (s+1)*512, :] = attnT^T @ wp."""
                for nt in range(4):
                    ysb = y_pool.tile([128, DIM], FP32, tag="y")
                    for og, ow in ((0, 512), (512, 256)):
                        psy = ps_small.tile([128, 512], FP32, tag="sm")
                        for cc in range(PAIRS):
                            nc.tensor.matmul(
                                psy[:, 0:ow],
                                attnT[:, cc, nt * 128 : nt * 128 + 128],
                                wp[:, cc, og : og + ow],
                                start=(cc == 0), stop=(cc == PAIRS - 1),
                            )
                        nc.vector.tensor_copy(out=ysb[:, og : og + ow], in_=psy[:, 0:ow])
                    row = s * 512 + nt * 128
                    nc.sync.dma_start(out=y_d.ap()[row : row + 128, :], in_=ysb)

            # chunking of the 16 kv blocks into score-psum chunks
            chunks = []
            j0 = 0
            while j0 < J:
                ln = min(CH, J - j0)
                chunks.append((j0, ln))
                j0 += ln

            qkv_pair(0)
            qkv_pair(1)

            attnT_tiles = {}
            for s in range(S):
                for hp in range(PAIRS):
                    if hp == 0:
                        attnT = attnt_pool.tile([128, PAIRS, 512], BF16, tag="attnT")
                        attnT_tiles[s] = attnT
                    attnT = attnT_tiles[s]

                    hA, hB = 2 * hp, 2 * hp + 1
                    # --- QK^T + exp, chunked over kv ---
                    expS = {0: [], 1: []}
                    for (j0, ln) in chunks:
                        for h2, base in ((0, 0), (1, 64)):
                            pss = ps_score.tile([128, 512 * CH], FP32, tag="sc")
                            h = 2 * hp + h2
                            for jj in range(ln):
                                j = j0 + jj
                                nc.tensor.matmul(
                                    pss[:, jj * 512 : jj * 512 + 512],
                                    qkT[base : base + 64, PAIRS + hp,
                                        j * 128 : j * 128 + 128],
                                    qkT[base : base + 64, hp,
                                        s * 512 : s * 512 + 512],
                                    start=True, stop=True,
                                    tile_position=(base, 0),
                                )
                            et = exps_pool.tile([128, 512 * CH], BF16, tag="e")
                            nc.scalar.activation(
                                out=et[:, : 512 * ln],
                                in_=pss[:, : 512 * ln],
                                func=AF.Exp,
                            )
                            expS[h2].append((j0, ln, et))

                    # pipelined heavy PE work while ACT runs exp:
                    if s == 0 and hp < PAIRS - 1:
                        qkv_pair(hp + 1)
                    if hp == 0 and s >= 1:
                        proj(s - 1, attnT_tiles.pop(s - 1))

                    # --- AV + divide ---
                    attn_pair = small_pool.tile([128, 4, 128], BF16, tag="ap")
                    for h2 in range(2):
                        h = 2 * hp + h2
                        pav = ps_small.tile([128, 512], FP32, tag="sm")
                        for i in range(4):
                            for (j0, ln, et) in expS[h2]:
                                for jj in range(ln):
                                    j = j0 + jj
                                    nc.tensor.matmul(
                                        pav[:, i * 128 : i * 128 + 65],
                                        et[:, jj * 512 + i * 128 : jj * 512 + i * 128 + 128],
                                        v[:, j, h, :],
                                        start=(j == 0), stop=(j == J - 1),
                                    )
                        pav4 = pav.rearrange("p (r c) -> p r c", r=4)
                        rsb = small_pool.tile([128, 4], FP32, tag="r")
                        nc.vector.reciprocal(out=rsb, in_=pav4[:, :, 64])
                        nc.vector.tensor_tensor(
                            attn_pair[:, :, h2 * 64 : h2 * 64 + 64],
                            pav4[:, :, 0:64],
                            rsb[:, :, None].to_broadcast((128, 4, 64)),
                            mybir.AluOpType.mult,
                        )

                    # --- transpose pair block into attnT ---
                    for i in range(4):
                        pst = ps_small.tile([128, 512], BF16, tag="sm")
                        nc.tensor.transpose(
                            pst[:, 0:128], attn_pair[:, i, :], ident
                        )
                        nc.vector.tensor_copy(
                            out=attnT[:, hp, i * 128 : i * 128 + 128],
                            in_=pst[:, 0:128],
                        )

            proj(S - 1, attnT_tiles.pop(S - 1))

    nc.compile()
    return nc


def _host_prep(x, w_qkv, w_proj):
    """Slice/transpose/cast inputs per core. Core c = 2*b + hg."""
    bf16 = ml_dtypes.bfloat16
    in_maps = []
    for c in range(8):
        b, hg = c // 2, c % 2
        r0 = 384 * hg
        wq = w_qkv[r0 : r0 + 384] * SCALE          # [384, 768] scaled q rows
        wk = w_qkv[768 + r0 : 768 + r0 + 384]
        wv = w_qkv[1536 + r0 : 1536 + r0 + 384]
        wqk = np.concatenate([wq, wk], axis=0)     # [768, 768]
        in_maps.append({
            "xt": np.ascontiguousarray(x[b].T).astype(bf16),
            "wqk": np.ascontiguousarray(wqk.T).astype(bf16),
            "wv": np.ascontiguousarray(wv.T).astype(bf16),
            "wp": np.ascontiguousarray(w_proj[:, r0 : r0 + 384].T).astype(bf16),
        })
    return in_maps


def _get_fn():
    """Build the Bass program and a cached jit callable over 8 cores."""
    if "fn" in _CACHED:
        return _CACHED["fn"]

    import jax
    from jax.sharding import Mesh, PartitionSpec
    from jax.experimental.shard_map import shard_map
    from concourse import bass2jax
    from concourse.bass2jax import _bass_exec_p, install_neuronx_cc_hook

    install_neuronx_cc_hook()
    nc = build_core_program()

    in_names = ["xt", "wqk", "wv", "wp"]
    out_avals = [jax.core.ShapedArray((N, DIM), np.float32)]
    partition_name = nc.partition_id_tensor.name if nc.partition_id_tensor else None

    def _body(xt, wqk, wv, wp, yzero):
        operands = [xt, wqk, wv, wp, yzero]
        names = in_names + ["y"]
        if nc.dbg_addr is not None:
            operands.append(np.zeros((1, 2), np.uint32))
            names.append(nc.dbg_addr.name)
        if partition_name is not None:
            operands.append(bass2jax.partition_id_tensor())
            names.append(partition_name)
        outs = _bass_exec_p.bind(
            *operands,
            out_avals=tuple(out_avals),
            in_names=tuple(names),
            out_names=("y",),
            lowering_input_output_aliases=(),
            sim_require_finite=True,
            sim_require_nnan=True,
            nc=nc,
        )
        return outs[0]

    devices = jax.devices()[:8]
    mesh = Mesh(np.asarray(devices), ("core",))
    fn = jax.jit(
        shard_map(
            _body, mesh=mesh,
            in_specs=(PartitionSpec("core"),) * 5,
            out_specs=PartitionSpec("core"),
            check_rep=False,
        ),
        keep_unused=True,
    )
    _CACHED["fn"] = fn
    return fn


def _run(in_maps):
    import jax

    fn = _get_fn()
    concat_in = [
        np.concatenate([m[name] for m in in_maps], axis=0)
        for name in ["xt", "wqk", "wv", "wp"]
    ]
    yzero = np.zeros((8 * N, DIM), np.float32)
    out = jax.block_until_ready(fn(*concat_in, yzero))
    return np.asarray(out).reshape(8, N, DIM)


def kernel(x, w_qkv, w_proj, b_proj):
    x = np.asarray(x, dtype=np.float32)
    w_qkv = np.asarray(w_qkv, dtype=np.float32)
    w_proj = np.asarray(w_proj, dtype=np.float32)
    b_proj = np.asarray(b_proj, dtype=np.float32)

    in_maps = _host_prep(x, w_qkv, w_proj)
    parts = _run(in_maps)

    y = np.empty((B, N, DIM), dtype=np.float32)
    for b in range(B):
        y[b] = parts[2 * b] + parts[2 * b + 1] + b_proj
    return y
